# revision 2
# baseline (speedup 1.0000x reference)
"""Differential multi-head self-attention on 8 Trainium2 NeuronCores, v2.

Sharding: core c handles batch b = c // 4 and heads {2*(c%4), 2*(c%4)+1}.
Each core emits per-head partial output projections (o_h @ Wo_h with rms_w
and (1 - lambda_init) folded into Wo on the host); the host sums partials
per batch and adds bo.

v3 device math per (b, h) — scores in [q, k] orientation, all-bf16 QKV:
  xT = x.T (PE transpose) kept as bf16
  QT[d,s], KT[d,s] = W.T @ xT  (bf16 matmuls, bf16 output)
  V[s,d] = x @ Wv              (bf16 matmuls)
  per q block of 128 rows (kw = causal width):
    S_half[q, k] = QT_half.T @ KT_half  (two bf16 matmuls per 512-span)
    diagonal 128x128 block gets -1e10 strictly-upper bias added in PSUM
    E_half = exp(S * scale), accum_out -> per-row sums s1, s2 (ACT)
    lamn = -lam * s1/s2;  F = E2 * lamn + E1   (one fused elementwise pass)
    FT = F.T per 128-block (PE transpose, bf16, packed PSUM drains)
    O[q, d] = sum_kc FT_kc.T @ V_kc            (single bf16 AV matmul chain)
    rr = exp(-0.5 * ln(eps*s1^2 + mean(O^2)))  (folds softmax 1/s1 into RMS)
    out_h[q, e] = rr * (O.T @ Wo_h')           (PE transpose + f32r matmul)
"""

import math

import numpy as np
import ml_dtypes

import concourse.bass as bass
import concourse.mybir as mybir
import concourse.tile as tile
from concourse import bacc
from concourse.bass_utils import run_bass_kernel_spmd
from concourse.hw_specs import get_activation_tables
from concourse.masks import make_identity

B, S, E, H, D = 2, 2048, 512, 8, 512
HALF = D // 2
HLOC = 2
NCORES = 8
NQB = S // 128      # 16 q blocks
NKC = S // 128      # 16 k chunks
NDC = D // 128
NEC = E // 128
NSC = S // 128
KSP = 512           # score span (free dim of score matmuls)
SCALE = 1.0 / float(np.sqrt(HALF))
SSCALE = SCALE
EPS = float(np.finfo(np.float32).eps)
LAMBDA_INIT = 0.8
MASKVAL = -1e10
RSQD = float(1.0 / np.sqrt(D))

f32 = mybir.dt.float32
f32r = mybir.dt.float32r
bf16 = mybir.dt.bfloat16
fp8 = mybir.dt.float8e4
AF = mybir.ActivationFunctionType
ALU = mybir.AluOpType
AXL = mybir.AxisListType
DR = mybir.MatmulPerfMode.DoubleRow

COMBINE_POOL = False  # gpsimd/Pool rejects scalar_tensor_tensor in walrus ISA check


def _build(repeat=1, unroll=1):  # noqa: C901
    nc = bacc.Bacc("TRN2", target_bir_lowering=False, debug=False)

    x_d = nc.dram_tensor("x", [S, E], f32, kind="ExternalInput")
    wq_d = nc.dram_tensor("wq", [HLOC, E, D], bf16, kind="ExternalInput")
    wk_d = nc.dram_tensor("wk", [HLOC, E, D], bf16, kind="ExternalInput")
    wv_d = nc.dram_tensor("wv", [HLOC, E, D], bf16, kind="ExternalInput")
    wo_d = nc.dram_tensor("wo", [HLOC, D, E], f32, kind="ExternalInput")
    lamneg_d = nc.dram_tensor("lamneg", [HLOC, 128, 1], f32, kind="ExternalInput")
    maskb_d = nc.dram_tensor("maskb", [128, 128], f32, kind="ExternalInput")
    out_d = nc.dram_tensor("out", [HLOC, S, E], f32, kind="ExternalOutput")
    iters_d = nc.dram_tensor("iters", [1, 1], f32, kind="ExternalOutput") if repeat > 1 else None

    act_sets = list(get_activation_tables(nc.m.arch).keys())
    nle_set = act_sets.index("natural_log_exp_and_others")

    with tile.TileContext(nc) as tc:
        with tc.tile_pool(name="cst", bufs=1) as cst, \
             tc.tile_pool(name="big", bufs=1) as big, \
             tc.tile_pool(name="wts", bufs=1) as wts, \
             tc.tile_pool(name="wts2", bufs=2) as wts2, \
             tc.tile_pool(name="epool", bufs=2) as epool, \
             tc.tile_pool(name="fpool", bufs=2) as fpool, \
             tc.tile_pool(name="ftpool", bufs=2) as ftpool, \
             tc.tile_pool(name="scr", bufs=2) as scr, \
             tc.tile_pool(name="scr2", bufs=2) as scr2, \
             tc.tile_pool(name="tiny", bufs=4) as tiny, \
             tc.tile_pool(name="pA", bufs=2, space="PSUM") as pA, \
             tc.tile_pool(name="pS", bufs=2, space="PSUM") as pS, \
             tc.tile_pool(name="pF", bufs=2, space="PSUM") as pF, \
             tc.tile_pool(name="pO", bufs=2, space="PSUM") as pO:

            nc.scalar.add_instruction(mybir.InstLoadActFuncSet(
                name=nc.get_next_instruction_name(),
                ins=[], outs=[], act_func_set_id=nle_set))

            ident = cst.tile([128, 128], f32, tag="ident")
            make_identity(nc, ident[:])
            identb = cst.tile([128, 128], bf16, tag="identb")
            nc.vector.tensor_copy(identb[:], ident[:])
            maskb_t = cst.tile([128, 128], f32, tag="maskb")
            nc.sync.dma_start(maskb_t[:], maskb_d.ap())
            lam_t = cst.tile([128, HLOC], f32, tag="lam")
            for h in range(HLOC):
                nc.sync.dma_start(lam_t[:, h:h + 1], lamneg_d.ap()[h])

            if repeat > 1:
                ctr = cst.tile([1, 1], f32, tag="ctr")
                nc.gpsimd.memset(ctr[:], 0.0)
            rep_ctx = tc.For_i(0, repeat, 1) if repeat > 1 else None
            if rep_ctx is not None:
                rep_ctx.__enter__()
                nc.vector.tensor_scalar_add(ctr[:], ctr[:], 1.0)

            def emit_body():
                # ---- phase A: x transposes -> xT_bf (bf16) + x8T (fp8)
                xT_bf = big.tile([128, NEC, S], bf16, tag="xTbf")
                for g in range(4):
                    xload = scr.tile([128, 4, E], f32, tag="xload")
                    nc.sync.dma_start(
                        xload[:],
                        x_d.ap()[g * 512:(g + 1) * 512, :].rearrange(
                            "(a p) e -> p a e", p=128))
                    for a in range(4):
                        st = g * 4 + a
                        tp = pA.tile([128, 4, 128], f32, tag="psA", name="xtp")
                        for ec in range(NEC):
                            nc.tensor.transpose(
                                tp[:, ec, :], xload[:, a, ec * 128:(ec + 1) * 128],
                                ident[:])
                        nc.vector.tensor_copy(
                            xT_bf[:, :, st * 128:(st + 1) * 128], tp[:])

                for h in range(HLOC):
                    wq_t = wts.tile([128, NEC, D], bf16, tag="wq")
                    wk_t = wts.tile([128, NEC, D], bf16, tag="wk")
                    wv_t = wts.tile([128, NEC, D], bf16, tag="wv")
                    wo_t = wts2.tile([128, NDC, E], f32r, tag="wo")
                    for ec in range(NEC):
                        nc.sync.dma_start(wq_t[:, ec, :], wq_d.ap()[h, ec * 128:(ec + 1) * 128, :])
                        nc.sync.dma_start(wk_t[:, ec, :], wk_d.ap()[h, ec * 128:(ec + 1) * 128, :])
                        nc.sync.dma_start(wv_t[:, ec, :], wv_d.ap()[h, ec * 128:(ec + 1) * 128, :])
                    for dc in range(NDC):
                        nc.sync.dma_start(wo_t[:, dc, :], wo_d.ap()[h, dc * 128:(dc + 1) * 128, :].bitcast(f32r))

                    # ---- projections
                    qt = big.tile([128, NDC, S], bf16, tag="qt")
                    kt = big.tile([128, NDC, S], bf16, tag="kt")
                    for dst, wt, drain in ((qt, wq_t, "dve"), (kt, wk_t, "act")):
                        for dc in range(NDC):
                            for sp in range(4):
                                pps = pA.tile([128, KSP], f32, tag="psA", name="pps")
                                for ec in range(NEC):
                                    nc.tensor.matmul(
                                        pps[:],
                                        wt[:, ec, dc * 128:(dc + 1) * 128],
                                        xT_bf[:, ec, sp * 512:(sp + 1) * 512],
                                        start=(ec == 0), stop=(ec == NEC - 1))
                                if drain == "dve":
                                    nc.vector.tensor_copy(
                                        dst[:, dc, sp * 512:(sp + 1) * 512], pps[:])
                                else:
                                    nc.scalar.activation(
                                        dst[:, dc, sp * 512:(sp + 1) * 512], pps[:],
                                        AF.Copy)

                    V = big.tile([128, NSC, D], bf16, tag="V")
                    for sc in range(NSC):
                        vps = pA.tile([128, D], f32, tag="psA", name="vps")
                        for ec in range(NEC):
                            nc.tensor.matmul(
                                vps[:],
                                xT_bf[:, ec, sc * 128:(sc + 1) * 128],
                                wv_t[:, ec, :],
                                start=(ec == 0), stop=(ec == NEC - 1))
                        nc.vector.tensor_copy(V[:, sc, :], vps[:])

                    # ---- q-block loop (software pipelined)
                    def emit_scores(qb):
                        nk = qb + 1
                        kw = nk * 128
                        nsp = (kw + KSP - 1) // KSP
                        E1 = epool.tile([128, S], bf16, tag="E1")
                        E2 = epool.tile([128, S], bf16, tag="E2")
                        sums = tiny.tile([128, 2, 4], f32, tag="sums")
                        for half in range(2):
                            Et = E1 if half == 0 else E2
                            for sp in range(nsp):
                                w = min(KSP, kw - sp * KSP)
                                sps = pS.tile([128, KSP], f32, tag="psS")
                                for j in range(2):
                                    dc = 2 * half + j
                                    nc.tensor.matmul(
                                        sps[:, :w],
                                        qt[:, dc, qb * 128:(qb + 1) * 128],
                                        kt[:, dc, sp * KSP:sp * KSP + w],
                                        start=(j == 0), stop=(j == 1))
                                if sp == nsp - 1:
                                    nc.vector.tensor_tensor(
                                        out=sps[:, w - 128:w], in0=sps[:, w - 128:w],
                                        in1=maskb_t[:], op=ALU.add)
                                nc.scalar.activation(
                                    Et[:, sp * KSP:sp * KSP + w], sps[:, :w],
                                    AF.Exp, scale=SSCALE,
                                    accum_out=sums[:, half, sp:sp + 1])
                        return (E1, E2, sums, nk, nsp)

                    def emit_combine(st):
                        E1, E2, sums, nk, nsp = st
                        kw = nk * 128
                        s12 = tiny.tile([128, 2], f32, tag="s12")
                        if nsp > 1:
                            nc.vector.tensor_reduce(
                                s12[:], sums[:, :, :nsp], axis=AXL.X, op=ALU.add)
                        else:
                            nc.vector.tensor_copy(s12[:], sums[:, :, 0])
                        rec2 = tiny.tile([128, 1], f32, tag="rec2")
                        nc.vector.reciprocal(rec2[:], s12[:, 1:2])
                        lamn = tiny.tile([128, 1], f32, tag="lamn")
                        nc.vector.tensor_tensor(
                            out=lamn[:], in0=s12[:, 0:1], in1=rec2[:], op=ALU.mult)
                        nc.vector.tensor_tensor(
                            out=lamn[:], in0=lamn[:], in1=lam_t[:, h:h + 1], op=ALU.mult)
                        bias = tiny.tile([128, 1], f32, tag="bias")
                        nc.vector.tensor_tensor(
                            out=bias[:], in0=s12[:, 0:1], in1=s12[:, 0:1], op=ALU.mult)
                        nc.vector.tensor_scalar_mul(bias[:], bias[:], EPS)
                        F = fpool.tile([128, S], bf16, tag="F")
                        eng = nc.gpsimd if COMBINE_POOL else nc.vector
                        eng.scalar_tensor_tensor(
                            out=F[:, :kw], in0=E2[:, :kw], scalar=lamn[:],
                            in1=E1[:, :kw], op0=ALU.mult, op1=ALU.add)
                        return (F, bias, nk)

                    def emit_ft_av(st):
                        F, bias, nk = st
                        FT = ftpool.tile([128, NKC, 128], bf16, tag="FT")
                        for g4 in range((nk + 3) // 4):
                            cnt = min(4, nk - 4 * g4)
                            ftp = pF.tile([128, 4, 128], bf16, tag="psF")
                            for j in range(cnt):
                                kc = 4 * g4 + j
                                nc.tensor.transpose(
                                    ftp[:, j, :], F[:, kc * 128:(kc + 1) * 128],
                                    identb[:])
                            nc.vector.tensor_copy(
                                FT[:, 4 * g4:4 * g4 + cnt, :], ftp[:, :cnt, :])
                        o_ps = pO.tile([128, D], f32, tag="psO")
                        for kc in range(nk):
                            nc.tensor.matmul(
                                o_ps[:], FT[:, kc, :], V[:, kc, :],
                                start=(kc == 0), stop=(kc == nk - 1))
                        return (o_ps, bias)

                    def emit_tail(st, qb):
                        o_ps, bias = st
                        O_sb = scr2.tile([128, D], f32, tag="Osb")
                        nc.vector.tensor_copy(O_sb[:], o_ps[:])
                        osq = scr2.tile([128, D], bf16, tag="osq")
                        ms = tiny.tile([128, 1], f32, tag="ms")
                        nc.scalar.activation(
                            osq[:], o_ps[:], AF.Square, scale=RSQD,
                            accum_out=ms[:])
                        lnm = tiny.tile([128, 1], f32, tag="lnm")
                        nc.scalar.activation(lnm[:], ms[:], AF.Ln, bias=bias[:])
                        rr = tiny.tile([128, 1], f32, tag="rr")
                        nc.scalar.activation(rr[:], lnm[:], AF.Exp, scale=-0.5)
                        otp = pA.tile([128, 4, 128], f32, tag="psA", name="otp")
                        for dc in range(NDC):
                            nc.tensor.transpose(
                                otp[:, dc, :], O_sb[:, dc * 128:(dc + 1) * 128],
                                ident[:])
                        ot = scr2.tile([128, NDC, 128], f32r, tag="ot")
                        nc.vector.tensor_copy(ot[:], otp[:])
                        out_ps = pA.tile([128, E], f32, tag="psA", name="outps")
                        for dc in range(NDC):
                            nc.tensor.matmul(
                                out_ps[:], ot[:, dc, :], wo_t[:, dc, :],
                                start=(dc == 0), stop=(dc == NDC - 1))
                        out_sb = scr2.tile([128, E], f32, tag="outsb")
                        nc.scalar.activation(out_sb[:], out_ps[:], AF.Copy, scale=rr[:])
                        nc.sync.dma_start(
                            out_d.ap()[h, qb * 128:(qb + 1) * 128, :], out_sb[:])

                    pend_c = None   # (combine result, qb)
                    pend_a = None   # (ft_av result, qb)
                    for qb in range(NQB):
                        s_st = emit_scores(qb)
                        c_st = emit_combine(s_st)
                        if pend_a is not None:
                            emit_tail(*pend_a)
                            pend_a = None
                        if pend_c is not None:
                            pend_a = (emit_ft_av(pend_c[0]), pend_c[1])
                            pend_c = None
                        pend_c = (c_st, qb)
                    if pend_a is not None:
                        emit_tail(*pend_a)
                    pend_a = (emit_ft_av(pend_c[0]), pend_c[1])
                    emit_tail(*pend_a)

            for _u in range(unroll):
                emit_body()

            if rep_ctx is not None:
                rep_ctx.__exit__(None, None, None)
                nc.sync.dma_start(iters_d.ap()[:], ctr[:])

    nc.compile()
    return nc


_CACHE = {}


def _get_program(repeat=1, unroll=1):
    key = (repeat, unroll)
    if key not in _CACHE:
        _CACHE[key] = _build(repeat=repeat, unroll=unroll)
    return _CACHE[key]


def make_in_maps(x, mask, Wq, bq, Wk, bk, Wv, bv, lq1, lk1, lq2, lk2,
                 lam_init_p, rms_w, Wo, bo, repeat=1, unroll=1):
    x = np.asarray(x, np.float32)
    mask = np.asarray(mask, bool)
    assert np.array_equal(mask, np.triu(np.ones((S, S), bool), 1)), \
        "kernel specialized for causal mask"
    Wq = np.asarray(Wq, np.float32)
    Wk = np.asarray(Wk, np.float32)
    Wv = np.asarray(Wv, np.float32)
    Wo = np.asarray(Wo, np.float32)
    for b_ in (bq, bk, bv):
        assert np.abs(np.asarray(b_)).max() == 0.0, "nonzero qkv bias unsupported"
    lam = (np.exp((np.asarray(lq1, np.float32) * np.asarray(lk1, np.float32)).sum(-1))
           - np.exp((np.asarray(lq2, np.float32) * np.asarray(lk2, np.float32)).sum(-1))
           + np.asarray(lam_init_p, np.float32))  # [H]
    woF = Wo.reshape(H, D, E) * ((1.0 - LAMBDA_INIT) * np.asarray(rms_w, np.float32))[:, :, None]

    wq_bf = Wq.astype(ml_dtypes.bfloat16)
    wk_bf = Wk.astype(ml_dtypes.bfloat16)
    wv_bf = Wv.astype(ml_dtypes.bfloat16)
    mb = np.where(np.triu(np.ones((128, 128), bool), 1), np.float32(MASKVAL),
                  np.float32(0.0))

    nc = _get_program(repeat=repeat, unroll=unroll)

    in_maps = []
    for c in range(NCORES):
        b = c // 4
        h0 = HLOC * (c % 4)
        lamneg = np.repeat((-lam[h0:h0 + HLOC]).astype(np.float32)[:, None, None], 128, axis=1)
        in_maps.append({
            "x": np.ascontiguousarray(x[b]),
            "wq": np.ascontiguousarray(wq_bf[h0:h0 + HLOC]),
            "wk": np.ascontiguousarray(wk_bf[h0:h0 + HLOC]),
            "wv": np.ascontiguousarray(wv_bf[h0:h0 + HLOC]),
            "wo": np.ascontiguousarray(woF[h0:h0 + HLOC]),
            "lamneg": np.ascontiguousarray(lamneg),
            "maskb": mb,
        })
    return nc, in_maps


def gather(results, bo):
    out = np.zeros((B, S, E), np.float32)
    for c in range(NCORES):
        out[c // 4] += results[c]["out"].sum(axis=0)
    out += np.asarray(bo, np.float32)[None, None, :]
    return out


def kernel(**inputs):
    nc, in_maps = make_in_maps(**inputs)
    res = run_bass_kernel_spmd(nc, in_maps, core_ids=list(range(NCORES)))
    return gather(res.results, inputs["bo"])


# revision 3
# speedup vs baseline: 1.0059x; 1.0059x over previous
"""Differential multi-head self-attention on 8 Trainium2 NeuronCores, v2.

Sharding: core c handles batch b = c // 4 and heads {2*(c%4), 2*(c%4)+1}.
Each core emits per-head partial output projections (o_h @ Wo_h with rms_w
and (1 - lambda_init) folded into Wo on the host); the host sums partials
per batch and adds bo.

v3 device math per (b, h) — scores in [q, k] orientation, all-bf16 QKV:
  xT = x.T (PE transpose) kept as bf16
  QT[d,s], KT[d,s] = W.T @ xT  (bf16 matmuls, bf16 output)
  V[s,d] = x @ Wv              (bf16 matmuls)
  per q block of 128 rows (kw = causal width):
    S_half[q, k] = QT_half.T @ KT_half  (two bf16 matmuls per 512-span)
    diagonal 128x128 block gets -1e10 strictly-upper bias added in PSUM
    E_half = exp(S * scale), accum_out -> per-row sums s1, s2 (ACT)
    lamn = -lam * s1/s2;  F = E2 * lamn + E1   (one fused elementwise pass)
    FT = F.T per 128-block (PE transpose, bf16, packed PSUM drains)
    O[q, d] = sum_kc FT_kc.T @ V_kc            (single bf16 AV matmul chain)
    rr = exp(-0.5 * ln(eps*s1^2 + mean(O^2)))  (folds softmax 1/s1 into RMS)
    out_h[q, e] = rr * (O.T @ Wo_h')           (PE transpose + f32r matmul)
"""

import math

import numpy as np
import ml_dtypes

import concourse.bass as bass
import concourse.mybir as mybir
import concourse.tile as tile
from concourse import bacc
from concourse.bass_utils import run_bass_kernel_spmd
from concourse.hw_specs import get_activation_tables
from concourse.masks import make_identity

B, S, E, H, D = 2, 2048, 512, 8, 512
HALF = D // 2
HLOC = 2
NCORES = 8
NQB = S // 128      # 16 q blocks
NKC = S // 128      # 16 k chunks
NDC = D // 128
NEC = E // 128
NSC = S // 128
KSP = 512           # score span (free dim of score matmuls)
WS = 64.0           # fp8 weight scale (half-2 path)
SCALE = 1.0 / float(np.sqrt(HALF))
SSCALE = SCALE
SSCALE2 = SCALE / (WS * WS)
EPS = float(np.finfo(np.float32).eps)
LAMBDA_INIT = 0.8
MASKVAL = -1e10
RSQD = float(1.0 / np.sqrt(D))

f32 = mybir.dt.float32
f32r = mybir.dt.float32r
bf16 = mybir.dt.bfloat16
fp8 = mybir.dt.float8e4
AF = mybir.ActivationFunctionType
ALU = mybir.AluOpType
AXL = mybir.AxisListType
DR = mybir.MatmulPerfMode.DoubleRow

COMBINE_POOL = False  # gpsimd/Pool rejects scalar_tensor_tensor in walrus ISA check


def _build(repeat=1, unroll=1):  # noqa: C901
    nc = bacc.Bacc("TRN2", target_bir_lowering=False, debug=False)

    x_d = nc.dram_tensor("x", [S, E], f32, kind="ExternalInput")
    wq_d = nc.dram_tensor("wq", [HLOC, E, HALF], bf16, kind="ExternalInput")
    wk_d = nc.dram_tensor("wk", [HLOC, E, HALF], bf16, kind="ExternalInput")
    wq8_d = nc.dram_tensor("wq8", [HLOC, E, HALF], fp8, kind="ExternalInput")
    wk8_d = nc.dram_tensor("wk8", [HLOC, E, HALF], fp8, kind="ExternalInput")
    wv_d = nc.dram_tensor("wv", [HLOC, E, D], bf16, kind="ExternalInput")
    wo_d = nc.dram_tensor("wo", [HLOC, D, E], f32, kind="ExternalInput")
    lamneg_d = nc.dram_tensor("lamneg", [HLOC, 128, 1], f32, kind="ExternalInput")
    maskb_d = nc.dram_tensor("maskb", [128, 128], f32, kind="ExternalInput")
    out_d = nc.dram_tensor("out", [HLOC, S, E], f32, kind="ExternalOutput")
    iters_d = nc.dram_tensor("iters", [1, 1], f32, kind="ExternalOutput") if repeat > 1 else None

    act_sets = list(get_activation_tables(nc.m.arch).keys())
    nle_set = act_sets.index("natural_log_exp_and_others")

    with tile.TileContext(nc) as tc:
        with tc.tile_pool(name="cst", bufs=1) as cst, \
             tc.tile_pool(name="big", bufs=1) as big, \
             tc.tile_pool(name="wts", bufs=1) as wts, \
             tc.tile_pool(name="wts2", bufs=2) as wts2, \
             tc.tile_pool(name="epool", bufs=2) as epool, \
             tc.tile_pool(name="fpool", bufs=2) as fpool, \
             tc.tile_pool(name="ftpool", bufs=2) as ftpool, \
             tc.tile_pool(name="scr", bufs=2) as scr, \
             tc.tile_pool(name="scr2", bufs=2) as scr2, \
             tc.tile_pool(name="tiny", bufs=4) as tiny, \
             tc.tile_pool(name="pA", bufs=2, space="PSUM") as pA, \
             tc.tile_pool(name="pS", bufs=2, space="PSUM") as pS, \
             tc.tile_pool(name="pF", bufs=2, space="PSUM") as pF, \
             tc.tile_pool(name="pO", bufs=2, space="PSUM") as pO:

            nc.scalar.add_instruction(mybir.InstLoadActFuncSet(
                name=nc.get_next_instruction_name(),
                ins=[], outs=[], act_func_set_id=nle_set))

            ident = cst.tile([128, 128], f32, tag="ident")
            make_identity(nc, ident[:])
            identb = cst.tile([128, 128], bf16, tag="identb")
            nc.vector.tensor_copy(identb[:], ident[:])
            maskb_t = cst.tile([128, 128], f32, tag="maskb")
            nc.sync.dma_start(maskb_t[:], maskb_d.ap())
            lam_t = cst.tile([128, HLOC], f32, tag="lam")
            for h in range(HLOC):
                nc.sync.dma_start(lam_t[:, h:h + 1], lamneg_d.ap()[h])

            if repeat > 1:
                ctr = cst.tile([1, 1], f32, tag="ctr")
                nc.gpsimd.memset(ctr[:], 0.0)
            rep_ctx = tc.For_i(0, repeat, 1) if repeat > 1 else None
            if rep_ctx is not None:
                rep_ctx.__enter__()
                nc.vector.tensor_scalar_add(ctr[:], ctr[:], 1.0)

            def emit_body():
                # ---- phase A: x transposes -> xT_bf (bf16) + x8T (fp8)
                xT_bf = big.tile([128, NEC, S], bf16, tag="xTbf")
                x8T = big.tile([128, NEC, S], fp8, tag="x8T")
                for g in range(4):
                    xload = scr.tile([128, 4, E], f32, tag="xload")
                    nc.sync.dma_start(
                        xload[:],
                        x_d.ap()[g * 512:(g + 1) * 512, :].rearrange(
                            "(a p) e -> p a e", p=128))
                    for a in range(4):
                        st = g * 4 + a
                        tp = pA.tile([128, 4, 128], f32, tag="psA", name="xtp")
                        for ec in range(NEC):
                            nc.tensor.transpose(
                                tp[:, ec, :], xload[:, a, ec * 128:(ec + 1) * 128],
                                ident[:])
                        nc.vector.tensor_copy(
                            xT_bf[:, :, st * 128:(st + 1) * 128], tp[:])
                        nc.scalar.activation(
                            x8T[:, :, st * 128:(st + 1) * 128], tp[:], AF.Copy)

                for h in range(HLOC):
                    wq_t = wts.tile([128, NEC, HALF], bf16, tag="wq")
                    wk_t = wts.tile([128, NEC, HALF], bf16, tag="wk")
                    wq8_t = wts.tile([128, NEC, HALF], fp8, tag="wq8")
                    wk8_t = wts.tile([128, NEC, HALF], fp8, tag="wk8")
                    wv_t = wts.tile([128, NEC, D], bf16, tag="wv")
                    wo_t = wts2.tile([128, NDC, E], f32r, tag="wo")
                    for ec in range(NEC):
                        nc.sync.dma_start(wq_t[:, ec, :], wq_d.ap()[h, ec * 128:(ec + 1) * 128, :])
                        nc.sync.dma_start(wk_t[:, ec, :], wk_d.ap()[h, ec * 128:(ec + 1) * 128, :])
                        nc.sync.dma_start(wq8_t[:, ec, :], wq8_d.ap()[h, ec * 128:(ec + 1) * 128, :])
                        nc.sync.dma_start(wk8_t[:, ec, :], wk8_d.ap()[h, ec * 128:(ec + 1) * 128, :])
                        nc.sync.dma_start(wv_t[:, ec, :], wv_d.ap()[h, ec * 128:(ec + 1) * 128, :])
                    for dc in range(NDC):
                        nc.sync.dma_start(wo_t[:, dc, :], wo_d.ap()[h, dc * 128:(dc + 1) * 128, :].bitcast(f32r))

                    # ---- projections
                    qt = big.tile([128, 2, S], bf16, tag="qt")
                    kt = big.tile([128, 2, S], bf16, tag="kt")
                    for dst, wt, drain in ((qt, wq_t, "dve"), (kt, wk_t, "act")):
                        for dc in range(2):
                            for sp in range(4):
                                pps = pA.tile([128, KSP], f32, tag="psA", name="pps")
                                for ec in range(NEC):
                                    nc.tensor.matmul(
                                        pps[:],
                                        wt[:, ec, dc * 128:(dc + 1) * 128],
                                        xT_bf[:, ec, sp * 512:(sp + 1) * 512],
                                        start=(ec == 0), stop=(ec == NEC - 1))
                                if drain == "dve":
                                    nc.vector.tensor_copy(
                                        dst[:, dc, sp * 512:(sp + 1) * 512], pps[:])
                                else:
                                    nc.scalar.activation(
                                        dst[:, dc, sp * 512:(sp + 1) * 512], pps[:],
                                        AF.Copy)
                    qt8 = big.tile([128, 2, S], fp8, tag="qt8")
                    kt8 = big.tile([128, 2, S], fp8, tag="kt8")
                    for dst, wt, drain in ((qt8, wq8_t, "dve"), (kt8, wk8_t, "act")):
                        for dc in range(2):
                            for sp in range(4):
                                pps = pA.tile([128, KSP], f32, tag="psA", name="pps")
                                for ecp in range(2):
                                    nc.tensor.matmul(
                                        pps[:],
                                        wt[:, 2 * ecp:2 * ecp + 2, dc * 128:(dc + 1) * 128],
                                        x8T[:, 2 * ecp:2 * ecp + 2, sp * 512:(sp + 1) * 512],
                                        start=(ecp == 0), stop=(ecp == 1),
                                        perf_mode=DR)
                                if drain == "dve":
                                    nc.vector.tensor_copy(
                                        dst[:, dc, sp * 512:(sp + 1) * 512], pps[:])
                                else:
                                    nc.scalar.activation(
                                        dst[:, dc, sp * 512:(sp + 1) * 512], pps[:],
                                        AF.Copy)

                    V = big.tile([128, NSC, D], bf16, tag="V")
                    for sc in range(NSC):
                        vps = pA.tile([128, D], f32, tag="psA", name="vps")
                        for ec in range(NEC):
                            nc.tensor.matmul(
                                vps[:],
                                xT_bf[:, ec, sc * 128:(sc + 1) * 128],
                                wv_t[:, ec, :],
                                start=(ec == 0), stop=(ec == NEC - 1))
                        nc.vector.tensor_copy(V[:, sc, :], vps[:])

                    # ---- q-block loop (software pipelined)
                    def emit_scores(qb):
                        nk = qb + 1
                        kw = nk * 128
                        nsp = (kw + KSP - 1) // KSP
                        E1 = epool.tile([128, S], bf16, tag="E1")
                        E2 = epool.tile([128, S], bf16, tag="E2")
                        sums = tiny.tile([128, 2, 4], f32, tag="sums")
                        for half in range(2):
                            Et = E1 if half == 0 else E2
                            for sp in range(nsp):
                                w = min(KSP, kw - sp * KSP)
                                sps = pS.tile([128, KSP], f32, tag="psS")
                                if half == 0:
                                    for j in range(2):
                                        nc.tensor.matmul(
                                            sps[:, :w],
                                            qt[:, j, qb * 128:(qb + 1) * 128],
                                            kt[:, j, sp * KSP:sp * KSP + w],
                                            start=(j == 0), stop=(j == 1))
                                else:
                                    nc.tensor.matmul(
                                        sps[:, :w],
                                        qt8[:, 0:2, qb * 128:(qb + 1) * 128],
                                        kt8[:, 0:2, sp * KSP:sp * KSP + w],
                                        start=True, stop=True, perf_mode=DR)
                                if sp == nsp - 1:
                                    nc.vector.tensor_tensor(
                                        out=sps[:, w - 128:w], in0=sps[:, w - 128:w],
                                        in1=maskb_t[:], op=ALU.add)
                                nc.scalar.activation(
                                    Et[:, sp * KSP:sp * KSP + w], sps[:, :w],
                                    AF.Exp, scale=(SSCALE if half == 0 else SSCALE2),
                                    accum_out=sums[:, half, sp:sp + 1])
                        return (E1, E2, sums, nk, nsp)

                    def emit_combine(st):
                        E1, E2, sums, nk, nsp = st
                        kw = nk * 128
                        s12 = tiny.tile([128, 2], f32, tag="s12")
                        if nsp > 1:
                            nc.vector.tensor_reduce(
                                s12[:], sums[:, :, :nsp], axis=AXL.X, op=ALU.add)
                        else:
                            nc.vector.tensor_copy(s12[:], sums[:, :, 0])
                        rec2 = tiny.tile([128, 1], f32, tag="rec2")
                        nc.vector.reciprocal(rec2[:], s12[:, 1:2])
                        lamn = tiny.tile([128, 1], f32, tag="lamn")
                        nc.vector.tensor_tensor(
                            out=lamn[:], in0=s12[:, 0:1], in1=rec2[:], op=ALU.mult)
                        nc.vector.tensor_tensor(
                            out=lamn[:], in0=lamn[:], in1=lam_t[:, h:h + 1], op=ALU.mult)
                        bias = tiny.tile([128, 1], f32, tag="bias")
                        nc.vector.tensor_tensor(
                            out=bias[:], in0=s12[:, 0:1], in1=s12[:, 0:1], op=ALU.mult)
                        nc.vector.tensor_scalar_mul(bias[:], bias[:], EPS)
                        F = fpool.tile([128, S], bf16, tag="F")
                        eng = nc.gpsimd if COMBINE_POOL else nc.vector
                        eng.scalar_tensor_tensor(
                            out=F[:, :kw], in0=E2[:, :kw], scalar=lamn[:],
                            in1=E1[:, :kw], op0=ALU.mult, op1=ALU.add)
                        return (F, bias, nk)

                    def emit_ft_av(st):
                        F, bias, nk = st
                        FT = ftpool.tile([128, NKC, 128], bf16, tag="FT")
                        for g4 in range((nk + 3) // 4):
                            cnt = min(4, nk - 4 * g4)
                            ftp = pF.tile([128, 4, 128], bf16, tag="psF")
                            for j in range(cnt):
                                kc = 4 * g4 + j
                                nc.tensor.transpose(
                                    ftp[:, j, :], F[:, kc * 128:(kc + 1) * 128],
                                    identb[:])
                            nc.vector.tensor_copy(
                                FT[:, 4 * g4:4 * g4 + cnt, :], ftp[:, :cnt, :])
                        o_ps = pO.tile([128, D], f32, tag="psO")
                        for kc in range(nk):
                            nc.tensor.matmul(
                                o_ps[:], FT[:, kc, :], V[:, kc, :],
                                start=(kc == 0), stop=(kc == nk - 1))
                        return (o_ps, bias)

                    def emit_tail(st, qb):
                        o_ps, bias = st
                        O_sb = scr2.tile([128, D], f32, tag="Osb")
                        nc.vector.tensor_copy(O_sb[:], o_ps[:])
                        osq = scr2.tile([128, D], bf16, tag="osq")
                        ms = tiny.tile([128, 1], f32, tag="ms")
                        nc.scalar.activation(
                            osq[:], o_ps[:], AF.Square, scale=RSQD,
                            accum_out=ms[:])
                        lnm = tiny.tile([128, 1], f32, tag="lnm")
                        nc.scalar.activation(lnm[:], ms[:], AF.Ln, bias=bias[:])
                        rr = tiny.tile([128, 1], f32, tag="rr")
                        nc.scalar.activation(rr[:], lnm[:], AF.Exp, scale=-0.5)
                        otp = pA.tile([128, 4, 128], f32, tag="psA", name="otp")
                        for dc in range(NDC):
                            nc.tensor.transpose(
                                otp[:, dc, :], O_sb[:, dc * 128:(dc + 1) * 128],
                                ident[:])
                        ot = scr2.tile([128, NDC, 128], f32r, tag="ot")
                        nc.vector.tensor_copy(ot[:], otp[:])
                        out_ps = pA.tile([128, E], f32, tag="psA", name="outps")
                        for dc in range(NDC):
                            nc.tensor.matmul(
                                out_ps[:], ot[:, dc, :], wo_t[:, dc, :],
                                start=(dc == 0), stop=(dc == NDC - 1))
                        out_sb = scr2.tile([128, E], f32, tag="outsb")
                        nc.scalar.activation(out_sb[:], out_ps[:], AF.Copy, scale=rr[:])
                        nc.sync.dma_start(
                            out_d.ap()[h, qb * 128:(qb + 1) * 128, :], out_sb[:])

                    pend_c = None   # (combine result, qb)
                    pend_a = None   # (ft_av result, qb)
                    for qb in range(NQB):
                        s_st = emit_scores(qb)
                        c_st = emit_combine(s_st)
                        if pend_a is not None:
                            emit_tail(*pend_a)
                            pend_a = None
                        if pend_c is not None:
                            pend_a = (emit_ft_av(pend_c[0]), pend_c[1])
                            pend_c = None
                        pend_c = (c_st, qb)
                    if pend_a is not None:
                        emit_tail(*pend_a)
                    pend_a = (emit_ft_av(pend_c[0]), pend_c[1])
                    emit_tail(*pend_a)

            for _u in range(unroll):
                emit_body()

            if rep_ctx is not None:
                rep_ctx.__exit__(None, None, None)
                nc.sync.dma_start(iters_d.ap()[:], ctr[:])

    nc.compile()
    return nc


_CACHE = {}


def _get_program(repeat=1, unroll=1):
    key = (repeat, unroll)
    if key not in _CACHE:
        _CACHE[key] = _build(repeat=repeat, unroll=unroll)
    return _CACHE[key]


def make_in_maps(x, mask, Wq, bq, Wk, bk, Wv, bv, lq1, lk1, lq2, lk2,
                 lam_init_p, rms_w, Wo, bo, repeat=1, unroll=1):
    x = np.asarray(x, np.float32)
    mask = np.asarray(mask, bool)
    assert np.array_equal(mask, np.triu(np.ones((S, S), bool), 1)), \
        "kernel specialized for causal mask"
    Wq = np.asarray(Wq, np.float32)
    Wk = np.asarray(Wk, np.float32)
    Wv = np.asarray(Wv, np.float32)
    Wo = np.asarray(Wo, np.float32)
    for b_ in (bq, bk, bv):
        assert np.abs(np.asarray(b_)).max() == 0.0, "nonzero qkv bias unsupported"
    lam = (np.exp((np.asarray(lq1, np.float32) * np.asarray(lk1, np.float32)).sum(-1))
           - np.exp((np.asarray(lq2, np.float32) * np.asarray(lk2, np.float32)).sum(-1))
           + np.asarray(lam_init_p, np.float32))  # [H]
    woF = Wo.reshape(H, D, E) * ((1.0 - LAMBDA_INIT) * np.asarray(rms_w, np.float32))[:, :, None]

    wq_bf = Wq[:, :, :HALF].astype(ml_dtypes.bfloat16)
    wk_bf = Wk[:, :, :HALF].astype(ml_dtypes.bfloat16)
    wq8 = (Wq[:, :, HALF:] * WS).astype(ml_dtypes.float8_e4m3)
    wk8 = (Wk[:, :, HALF:] * WS).astype(ml_dtypes.float8_e4m3)
    wv_bf = Wv.astype(ml_dtypes.bfloat16)
    mb = np.where(np.triu(np.ones((128, 128), bool), 1), np.float32(MASKVAL),
                  np.float32(0.0))

    nc = _get_program(repeat=repeat, unroll=unroll)

    in_maps = []
    for c in range(NCORES):
        b = c // 4
        h0 = HLOC * (c % 4)
        lamneg = np.repeat((-lam[h0:h0 + HLOC]).astype(np.float32)[:, None, None], 128, axis=1)
        in_maps.append({
            "x": np.ascontiguousarray(x[b]),
            "wq": np.ascontiguousarray(wq_bf[h0:h0 + HLOC]),
            "wk": np.ascontiguousarray(wk_bf[h0:h0 + HLOC]),
            "wq8": np.ascontiguousarray(wq8[h0:h0 + HLOC]),
            "wk8": np.ascontiguousarray(wk8[h0:h0 + HLOC]),
            "wv": np.ascontiguousarray(wv_bf[h0:h0 + HLOC]),
            "wo": np.ascontiguousarray(woF[h0:h0 + HLOC]),
            "lamneg": np.ascontiguousarray(lamneg),
            "maskb": mb,
        })
    return nc, in_maps


def gather(results, bo):
    out = np.zeros((B, S, E), np.float32)
    for c in range(NCORES):
        out[c // 4] += results[c]["out"].sum(axis=0)
    out += np.asarray(bo, np.float32)[None, None, :]
    return out


def kernel(**inputs):
    nc, in_maps = make_in_maps(**inputs)
    res = run_bass_kernel_spmd(nc, in_maps, core_ids=list(range(NCORES)))
    return gather(res.results, inputs["bo"])
